# revision 1
# baseline (speedup 1.0000x reference)
"""Causal multi-head self-attention with RoPE on 8 Trainium2 NeuronCores.

Tensor-parallel over heads: each core handles 2 of 16 heads end-to-end
(QKV projection, RoPE, causal softmax attention, output projection with its
W_o row block). Host sums the 8 rank-128 partial outputs.

Device layouts (per core, per batch b):
  Q^T/K^T [128, 2048] f32r: rows = [h0:(even d | odd d), h1:(even d | odd d)]
  V       [128, 16, 2, 65] f32r: [t-part, t-block, head, (64 d | ones)]
  S^T     per (kb, q-chunk): rows k, cols q (causal: q >= kb*128); exp on ACT
  out^T accum via [V|1] lhsT -> row 64 = softmax denominators
All matmuls in fp32r (11-bit mantissa inputs, fp32 accumulate).
"""
import numpy as np
from contextlib import ExitStack

import concourse.bass as bass
import concourse.tile as tile
from concourse import bacc, mybir
from concourse.bass_utils import run_bass_kernel_spmd

F32 = mybir.dt.float32
F32R = mybir.dt.float32r
AF = mybir.ActivationFunctionType
BF16 = mybir.dt.bfloat16

D, H, DK, T, B = 1024, 16, 64, 2048, 4
NCORES, HPC = 8, 2
NT = B * T
ROPE_THETA = 10000.0
_BUILT = {}


def _build_nc(reps=1, phases="ABC", tp=False):
    nc = bacc.Bacc("TRN2", target_bir_lowering=False, debug=False,
                   num_devices=NCORES)
    xT = nc.dram_tensor("xT", [D, NT], F32R, kind="ExternalInput").ap()
    wq = nc.dram_tensor("wq", [D, 128], F32R, kind="ExternalInput").ap()
    wk = nc.dram_tensor("wk", [D, 128], F32R, kind="ExternalInput").ap()
    wv = nc.dram_tensor("wv", [D, 128], F32R, kind="ExternalInput").ap()
    wo = nc.dram_tensor("wo", [128, D], F32R, kind="ExternalInput").ap()
    cc = nc.dram_tensor("cc", [128, T], F32, kind="ExternalInput").ap()
    ss = nc.dram_tensor("ss", [128, T], F32, kind="ExternalInput").ap()
    trib = nc.dram_tensor("trib", [128, 128], BF16, kind="ExternalInput").ap()
    idb = nc.dram_tensor("idb", [128, 128], BF16, kind="ExternalInput").ap()
    ident = nc.dram_tensor("ident", [128, 128], F32R, kind="ExternalInput").ap()
    ones = nc.dram_tensor("ones", [128, 32], F32R, kind="ExternalInput").ap()
    out = nc.dram_tensor("out", [NT, D], F32, kind="ExternalOutput").ap()

    x3 = xT.rearrange("(dt p) n -> p dt n", p=128)   # [128, 8, NT]
    wq3 = wq.rearrange("(dt p) m -> p dt m", p=128)  # [128, 8, 128]
    wk3 = wk.rearrange("(dt p) m -> p dt m", p=128)
    wv3 = wv.rearrange("(dt p) m -> p dt m", p=128)

    with tile.TileContext(nc) as tc, ExitStack() as ctx:
        consts = ctx.enter_context(tc.tile_pool(name="consts", bufs=1))
        wpool = ctx.enter_context(tc.tile_pool(name="wpool", bufs=1))
        xin = ctx.enter_context(tc.tile_pool(name="xin", bufs=2))
        qkv = ctx.enter_context(tc.tile_pool(name="qkv", bufs=2))
        rope = ctx.enter_context(tc.tile_pool(name="rope", bufs=1))
        vtp = ctx.enter_context(tc.tile_pool(name="vtp", bufs=2))
        ptp = ctx.enter_context(tc.tile_pool(name="ptp", bufs=2))
        nrm = ctx.enter_context(tc.tile_pool(name="nrm", bufs=1))
        osb = ctx.enter_context(tc.tile_pool(name="osb", bufs=2))
        ps_s = ctx.enter_context(tc.tile_pool(name="ps_s", bufs=2, space="PSUM"))
        ps_av = ctx.enter_context(tc.tile_pool(name="ps_av", bufs=2, space="PSUM"))
        ps_mix = ctx.enter_context(tc.tile_pool(name="ps_mix", bufs=2, space="PSUM"))

        w_q = wpool.tile([128, 8, 128], F32R)
        w_k = wpool.tile([128, 8, 128], F32R)
        w_v = wpool.tile([128, 8, 128], F32R)
        w_o = wpool.tile([128, D], F32R)
        c_cc = consts.tile([128, T], F32)
        c_ss = consts.tile([128, T], F32)
        c_trib = consts.tile([128, 128], BF16)
        c_idb = consts.tile([128, 128], BF16)
        c_id = consts.tile([128, 128], F32R)
        nc.sync.dma_start(w_q[:], wq3)
        nc.sync.dma_start(w_k[:], wk3)
        nc.sync.dma_start(w_v[:], wv3)
        nc.sync.dma_start(w_o[:], wo)
        nc.sync.dma_start(c_cc[:], cc)
        nc.sync.dma_start(c_ss[:], ss)
        nc.sync.dma_start(c_trib[:], trib)
        nc.sync.dma_start(c_idb[:], idb)
        nc.sync.dma_start(c_id[:], ident)

        import itertools
        for rep, b in itertools.product(range(reps), range(B)):
            # ---------------- Phase A: QKV^T projection + RoPE + V transpose
            qt = qkv.tile([128, T], F32R, tag="qt")
            kt = qkv.tile([128, T], F32R, tag="kt")
            vsb = qkv.tile([128, 16, 2, 65], F32R, tag="vsb")
            bq = rope.tile([128, T], F32R, tag="bq")
            bk = rope.tile([128, T], F32R, tag="bk")
            bsq = rope.tile([128, T], F32R, tag="bsq")
            bsk = rope.tile([128, T], F32R, tag="bsk")
            nc.sync.dma_start(vsb[:, :, :, 64:65],
                              ones.rearrange("p (g h o) -> p g h o", g=16, h=2))
            for tb in range(4):
                col0 = b * T + tb * 512
                lt = tb * 512
                xt = xin.tile([128, 8, 512], F32R, tag="xt")
                nc.sync.dma_start(xt[:], x3[:, :, col0:col0 + 512])
                for which, w_sb, dest, bdest in (
                        ("q", w_q, qt, bq), ("k", w_k, kt, bk),
                        ("v", w_v, None, None)):
                    psA = ps_mix.tile([128, 512], F32, tag="mix")
                    for dt_i in range(8):
                        nc.tensor.matmul(psA[:], w_sb[:, dt_i, :], xt[:, dt_i, :],
                                         start=(dt_i == 0), stop=(dt_i == 7))
                    if which == "v":
                        vt = vtp.tile([128, 512], F32R, tag="vt")
                        nc.scalar.activation(vt[:], psA[:], AF.Copy)
                        for s in range(4):
                            g = tb * 4 + s
                            ptr = ps_mix.tile([128, 128], F32R, tag="mix")
                            nc.tensor.transpose(ptr[:], vt[:, s * 128:(s + 1) * 128],
                                                c_id[:])
                            dst = vsb[:, g, :, 0:64]
                            src = ptr[:].rearrange("p (h d) -> p h d", h=2)
                            if s % 2 == 0:
                                nc.vector.tensor_copy(dst, src)
                            else:
                                nc.scalar.activation(dst, src, AF.Copy)
                    else:
                        nc.vector.tensor_mul(dest[:, lt:lt + 512], psA[:],
                                             c_cc[:, lt:lt + 512])
                        nc.vector.tensor_mul(bdest[:, lt:lt + 512], psA[:],
                                             c_ss[:, lt:lt + 512])
                if tb in (1, 3):
                    # RoPE cross-term for finished half: swap 32-row halves
                    # within each head block, add. Done per column half so
                    # attention on early q/k columns can start sooner.
                    hlo = (tb - 1) * 512
                    for bt, bst, dest in ((bq, bsq, qt), (bk, bsk, kt)):
                        for hh in range(2):
                            r0 = hh * 64
                            nc.sync.dma_start(
                                bst[r0 + 32:r0 + 64, hlo:hlo + 1024],
                                bt[r0:r0 + 32, hlo:hlo + 1024])
                            nc.sync.dma_start(
                                bst[r0:r0 + 32, hlo:hlo + 1024],
                                bt[r0 + 32:r0 + 64, hlo:hlo + 1024])
                        nc.vector.tensor_add(dest[:, hlo:hlo + 1024],
                                             dest[:, hlo:hlo + 1024],
                                             bst[:, hlo:hlo + 1024])

            if "B" not in phases:
                continue
            # ---------------- Phase B: causal attention, heads packed on
            # PE row-groups (h0 rows 0-63, h1 rows 64-127 via tile_position)
            attn = qkv.tile([128, T], F32R, tag="attnT")
            for qc in range(4):
                av0 = ps_av.tile([65, 512], F32, tag="av")
                av1 = ps_av.tile([65, 512], F32, tag="av")
                for kb in range(4 * qc + 4):
                    k0 = kb * 128
                    q0 = max(qc * 512, k0)
                    q1 = qc * 512 + 512
                    n = q1 - q0
                    diag = (q0 == k0)
                    sps = ps_s.tile([128, 1024], F32, tag="sps")
                    for h in range(2):
                        nc.tensor.matmul(
                            sps[:, h * 512:h * 512 + n],
                            kt[h * 64:(h + 1) * 64, k0:k0 + 128],
                            qt[h * 64:(h + 1) * 64, q0:q1],
                            start=True, stop=not diag,
                            tile_position=(h * 64, 0) if tp else None,
                            skip_group_check=True)
                    if diag:  # causal mask: -1e30 upper triangle via PE
                        for h in range(2):
                            nc.tensor.matmul(
                                sps[:, h * 512:h * 512 + 128],
                                c_idb[:], c_trib[:],
                                start=False, stop=True,
                                skip_group_check=True)
                    pt = ptp.tile([128, 2, 512], F32R, tag="pt")
                    nc.scalar.activation(
                        pt[:, :, 0:n],
                        sps[:].rearrange("p (h f) -> p h f", h=2)[:, :, 0:n],
                        AF.Exp)
                    for h, av in ((0, av0), (1, av1)):
                        nc.tensor.matmul(
                            av[:, q0 - qc * 512:512],
                            vsb[:, kb, h, :], pt[:, h, 0:n],
                            start=(kb == 0), stop=(kb == 4 * qc + 3),
                            skip_group_check=True)
                # end of pass: normalize both heads, free PSUM quickly
                for h, av in ((0, av0), (1, av1)):
                    avc = nrm.tile([65, 512], F32, tag="avc")
                    nc.vector.tensor_copy(avc[0:65, :], av[0:65, :])
                    rc = nrm.tile([128, 512], F32, tag="rc")
                    nc.vector.reciprocal(rc[64:65, :], avc[64:65, :])
                    nc.gpsimd.dma_start(rc[0:1, :], rc[64:65, :])
                    rb = nrm.tile([64, 512], F32, tag="rb")
                    nc.gpsimd.partition_broadcast(rb[0:64, :], rc[0:1, :])
                    qlo = qc * 512
                    if h == 0:
                        nc.vector.tensor_mul(attn[0:64, qlo:qlo + 512],
                                             avc[0:64, :], rb[0:64, :])
                    else:
                        a1 = nrm.tile([64, 512], F32R, tag="a1")
                        nc.vector.tensor_mul(a1[0:64, :], avc[0:64, :],
                                             rb[0:64, :])
                        nc.gpsimd.dma_start(attn[64:128, qlo:qlo + 512],
                                            a1[0:64, :])

            # ---------------- Phase C: output projection (W_o row block)
            if "C" not in phases:
                continue
            for tp in range(8):
                o_sb = osb.tile([128, 2, D], F32, tag="osb")
                for s in range(2):
                    tt = tp * 2 + s
                    for h5 in range(2):
                        pso_t = ps_mix.tile([128, 512], F32, tag="mix")
                        pso = pso_t[:]
                        nc.tensor.matmul(pso,
                                         attn[:, tt * 128:(tt + 1) * 128],
                                         w_o[:, h5 * 512:(h5 + 1) * 512],
                                         start=True, stop=True)
                        dst = o_sb[:, s, h5 * 512:(h5 + 1) * 512]
                        if (s + h5) % 2 == 0:
                            nc.vector.tensor_copy(dst, pso)
                        else:
                            nc.scalar.activation(dst, pso, AF.Copy)
                row = b * T + tp * 256
                nc.sync.dma_start(
                    out[row:row + 256, :].rearrange("(s p) f -> p s f", p=128),
                    o_sb[:])

    nc.compile()
    return nc


def _host_prep(x, W_qkv, W_o, token_positions):
    x = np.ascontiguousarray(np.asarray(x, np.float32))
    W_qkv = np.asarray(W_qkv, np.float32)
    W_o = np.asarray(W_o, np.float32)
    pos = np.asarray(token_positions, np.float64)
    xT = np.ascontiguousarray(x.reshape(NT, D).T)
    i = np.arange(32)
    inv = 1.0 / (ROPE_THETA ** (2 * i / DK))
    ang = pos[None, :] * inv[:, None]
    cos, sin = np.cos(ang), np.sin(ang)
    CC = np.tile(cos, (4, 1)).astype(np.float32)
    SS = np.concatenate([sin, -sin, sin, -sin], 0).astype(np.float32)
    import ml_dtypes
    trib = np.where(np.arange(128)[:, None] <= np.arange(128)[None, :],
                    0.0, -1e30).astype(ml_dtypes.bfloat16)
    idb = np.eye(128).astype(ml_dtypes.bfloat16)
    ident = np.eye(128, dtype=np.float32)
    in_maps = []
    for c in range(NCORES):
        qcols, vcols = [], []
        for h in range(HPC):
            hh = HPC * c + h
            for half in range(2):
                qcols.extend(hh * DK + 2 * ii + half for ii in range(32))
            vcols.extend(hh * DK + d for d in range(DK))
        qcols = np.array(qcols)
        vcols = np.array(vcols)
        in_maps.append({
            "xT": xT,
            "wq": np.ascontiguousarray(W_qkv[:, 0 * D + qcols]),
            "wk": np.ascontiguousarray(W_qkv[:, 1 * D + qcols] / 8.0),
            "wv": np.ascontiguousarray(W_qkv[:, 2 * D + vcols]),
            "wo": np.ascontiguousarray(W_o[vcols, :]),
            "cc": CC, "ss": SS, "trib": trib, "idb": idb, "ident": ident,
            "ones": np.ones((128, 32), np.float32),
        })
    return in_maps


def kernel(x, W_qkv, W_o, token_positions, _trace=False):
    in_maps = _host_prep(x, W_qkv, W_o, token_positions)
    if "nc" not in _BUILT:
        _BUILT["nc"] = _build_nc()
    res = run_bass_kernel_spmd(_BUILT["nc"], in_maps,
                               core_ids=list(range(NCORES)), trace=_trace)
    _BUILT["last_result"] = res
    total = np.zeros((NT, D), np.float32)
    for r in res.results:
        total += r["out"]
    return total.reshape(B, T, D)



# revision 8
# speedup vs baseline: 1.0080x; 1.0080x over previous
"""Causal multi-head self-attention with RoPE on 8 Trainium2 NeuronCores.

Tensor-parallel over heads: each core handles 2 of 16 heads end-to-end
(QKV projection, RoPE, causal softmax attention, output projection with its
W_o row block). Host sums the 8 rank-128 partial outputs.

Device layouts (per core, per batch b):
  Q^T/K^T [128, 2048] bf16: rows = [h0:(even d | odd d), h1:(even d | odd d)]
  V       [128, 16, 2, 65] bf16: [t-part, t-block, head, (64 d | ones)]
  S^T     per (kb, q-chunk): rows k, cols q (causal: q >= kb*128); exp on ACT
  out^T accum via [V|1] lhsT -> row 64 = softmax denominators
All matmuls in bf16 (fp32 accumulate in PSUM); FWL fast weight loads apply.
"""
import numpy as np
from contextlib import ExitStack

import concourse.bass as bass
import concourse.tile as tile
from concourse import bacc, mybir
from concourse.bass_utils import run_bass_kernel_spmd

F32 = mybir.dt.float32
BF16 = mybir.dt.bfloat16
AF = mybir.ActivationFunctionType

D, H, DK, T, B = 1024, 16, 64, 2048, 4
NCORES, HPC = 8, 2
NT = B * T
ROPE_THETA = 10000.0
_BUILT = {}


def _build_nc():
    nc = bacc.Bacc("TRN2", target_bir_lowering=False, debug=False,
                   num_devices=NCORES)
    xT = nc.dram_tensor("xT", [D, NT], BF16, kind="ExternalInput").ap()
    wq = nc.dram_tensor("wq", [D, 128], BF16, kind="ExternalInput").ap()
    wk = nc.dram_tensor("wk", [D, 128], BF16, kind="ExternalInput").ap()
    wv = nc.dram_tensor("wv", [D, 128], BF16, kind="ExternalInput").ap()
    wo = nc.dram_tensor("wo", [128, D], BF16, kind="ExternalInput").ap()
    cc = nc.dram_tensor("cc", [128, T], F32, kind="ExternalInput").ap()
    ss = nc.dram_tensor("ss", [128, T], F32, kind="ExternalInput").ap()
    trib = nc.dram_tensor("trib", [128, 128], BF16, kind="ExternalInput").ap()
    idb = nc.dram_tensor("idb", [128, 128], BF16, kind="ExternalInput").ap()
    ones = nc.dram_tensor("ones", [128, 32], BF16, kind="ExternalInput").ap()
    out = nc.dram_tensor("out", [NT, D], BF16, kind="ExternalOutput").ap()

    x3 = xT.rearrange("(dt p) n -> p dt n", p=128)   # [128, 8, NT]
    wq3 = wq.rearrange("(dt p) m -> p dt m", p=128)  # [128, 8, 128]
    wk3 = wk.rearrange("(dt p) m -> p dt m", p=128)
    wv3 = wv.rearrange("(dt p) m -> p dt m", p=128)

    with tile.TileContext(nc) as tc, ExitStack() as ctx:
        consts = ctx.enter_context(tc.tile_pool(name="consts", bufs=1))
        xin = ctx.enter_context(tc.tile_pool(name="xin", bufs=2))
        qkv = ctx.enter_context(tc.tile_pool(name="qkv", bufs=2))
        rope = ctx.enter_context(tc.tile_pool(name="rope", bufs=2))
        vtp = ctx.enter_context(tc.tile_pool(name="vtp", bufs=2))
        ptp = ctx.enter_context(tc.tile_pool(name="ptp", bufs=2))
        nrm = ctx.enter_context(tc.tile_pool(name="nrm", bufs=2))
        osb = ctx.enter_context(tc.tile_pool(name="osb", bufs=2))
        ps_s = ctx.enter_context(tc.tile_pool(name="ps_s", bufs=2, space="PSUM"))
        ps_av = ctx.enter_context(tc.tile_pool(name="ps_av", bufs=2, space="PSUM"))
        ps_mix = ctx.enter_context(tc.tile_pool(name="ps_mix", bufs=2, space="PSUM"))

        w_q = consts.tile([128, 8, 128], BF16)
        w_k = consts.tile([128, 8, 128], BF16)
        w_v = consts.tile([128, 8, 128], BF16)
        w_o = consts.tile([128, D], BF16)
        c_cc = consts.tile([128, T], F32)
        c_ss = consts.tile([128, T], F32)
        c_trib = consts.tile([128, 128], BF16)
        c_idb = consts.tile([128, 128], BF16)
        nc.sync.dma_start(w_q[:], wq3)
        nc.sync.dma_start(w_k[:], wk3)
        nc.sync.dma_start(w_v[:], wv3)
        nc.sync.dma_start(w_o[:], wo)
        nc.sync.dma_start(c_cc[:], cc)
        nc.sync.dma_start(c_ss[:], ss)
        nc.sync.dma_start(c_trib[:], trib)
        nc.sync.dma_start(c_idb[:], idb)

        for b in range(B):
            # ---------------- Phase A: QKV^T projection + RoPE + V transpose
            xt = xin.tile([128, 8, T], BF16, tag="xt")
            nc.sync.dma_start(xt[:], x3[:, :, b * T:(b + 1) * T])
            qt = qkv.tile([128, T], BF16, tag="qt")
            kt = qkv.tile([128, T], BF16, tag="kt")
            vsb = qkv.tile([128, 16, 2, 65], BF16, tag="vsb")
            nc.sync.dma_start(vsb[:, :, :, 64:65],
                              ones.rearrange("p (g h o) -> p g h o", g=16, h=2))
            for half in range(2):
                hlo = half * 1024
                bq = rope.tile([128, 1024], BF16, tag="bq")
                bk = rope.tile([128, 1024], BF16, tag="bk")
                bsq = rope.tile([128, 1024], BF16, tag="bsq")
                bsk = rope.tile([128, 1024], BF16, tag="bsk")
                for sub in range(2):
                    lt = hlo + sub * 512
                    ls = sub * 512
                    for which, w_sb, dest, bdest in (
                            ("q", w_q, qt, bq), ("k", w_k, kt, bk),
                            ("v", w_v, None, None)):
                        psA = ps_mix.tile([128, 512], F32, tag="mix")
                        for dt_i in range(8):
                            nc.tensor.matmul(psA[:], w_sb[:, dt_i, :],
                                             xt[:, dt_i, lt:lt + 512],
                                             start=(dt_i == 0), stop=(dt_i == 7))
                        if which == "v":
                            vt = vtp.tile([128, 512], BF16, tag="vt")
                            nc.scalar.activation(vt[:], psA[:], AF.Copy)
                            for s in range(4):
                                g = (lt // 128) + s
                                ptr = ps_mix.tile([128, 128], BF16, tag="mix")
                                nc.tensor.transpose(
                                    ptr[:], vt[:, s * 128:(s + 1) * 128],
                                    c_idb[:])
                                dst = vsb[:, g, :, 0:64]
                                src = ptr[:].rearrange("p (h d) -> p h d", h=2)
                                if s % 2 == 0:
                                    nc.vector.tensor_copy(dst, src)
                                else:
                                    nc.scalar.activation(dst, src, AF.Copy)
                        else:
                            nc.vector.tensor_mul(dest[:, lt:lt + 512], psA[:],
                                                 c_cc[:, lt:lt + 512])
                            nc.vector.tensor_mul(bdest[:, ls:ls + 512], psA[:],
                                                 c_ss[:, lt:lt + 512])
                # RoPE cross-term: swap 32-row halves within each head block,
                # add into q^T/k^T. Swap DMAs go on the scalar queue so they
                # don't sit behind x loads on the sync queue.
                for bt, bst, dest in ((bq, bsq, qt), (bk, bsk, kt)):
                    for hh in range(2):
                        r0 = hh * 64
                        nc.scalar.dma_start(bst[r0 + 32:r0 + 64, :],
                                            bt[r0:r0 + 32, :])
                        nc.scalar.dma_start(bst[r0:r0 + 32, :],
                                            bt[r0 + 32:r0 + 64, :])
                    nc.vector.tensor_add(dest[:, hlo:hlo + 1024],
                                         dest[:, hlo:hlo + 1024], bst[:])

            # ---------------- Phase B: causal attention, heads packed on
            # PE row-groups (h0 rows 0-63, h1 rows 64-127)
            attn = qkv.tile([128, T], BF16, tag="attnT")
            for qc in range(4):
                av0 = ps_av.tile([65, 512], F32, tag="av")
                av1 = ps_av.tile([65, 512], F32, tag="av")
                for kb in range(4 * qc + 4):
                    k0 = kb * 128
                    q0 = max(qc * 512, k0)
                    q1 = qc * 512 + 512
                    n = q1 - q0
                    diag = (q0 == k0)
                    sps = ps_s.tile([128, 1024], F32, tag="sps")
                    for h in range(2):
                        nc.tensor.matmul(
                            sps[:, h * 512:h * 512 + n],
                            kt[h * 64:(h + 1) * 64, k0:k0 + 128],
                            qt[h * 64:(h + 1) * 64, q0:q1],
                            start=True, stop=not diag,
                            skip_group_check=True)
                    if diag:  # causal mask: -1e30 upper triangle via PE
                        for h in range(2):
                            nc.tensor.matmul(
                                sps[:, h * 512:h * 512 + 128],
                                c_idb[:], c_trib[:],
                                start=False, stop=True,
                                skip_group_check=True)
                    pt = ptp.tile([128, 2, 512], BF16, tag="pt")
                    nc.scalar.activation(
                        pt[:, :, 0:n],
                        sps[:].rearrange("p (h f) -> p h f", h=2)[:, :, 0:n],
                        AF.Exp)
                    for h, av in ((0, av0), (1, av1)):
                        nc.tensor.matmul(
                            av[:, q0 - qc * 512:512],
                            vsb[:, kb, h, :], pt[:, h, 0:n],
                            start=(kb == 0), stop=(kb == 4 * qc + 3),
                            skip_group_check=True)
                # normalize both heads; fast approx reciprocal (err ~51 ULP,
                # plenty for softmax denominators), broadcast on gpsimd
                qlo = qc * 512
                for h, av in ((0, av0), (1, av1)):
                    # denom row -> [128,4] via DMA so the exact reciprocal
                    # runs on 128 partitions (cost ~ cols), then back to a
                    # [1,512] row for the partition broadcast.
                    rcps = nrm.tile([128, 2, 512], F32, tag="rcps")
                    rsq = nrm.tile([128, 8], F32, tag="rsq")
                    nc.vector.tensor_copy(rcps[64:65, 0, :], av[64:65, :])
                    nc.gpsimd.dma_start(rsq[:, 0:4], rcps[64:65, 0, :])
                    nc.vector.reciprocal(rsq[:, 4:8], rsq[:, 0:4])
                    nc.gpsimd.dma_start(rcps[0:1, 1, :], rsq[:, 4:8])
                    rb = nrm.tile([64, 512], F32, tag="rb")
                    nc.gpsimd.partition_broadcast(rb[0:64, :], rcps[0:1, 1, :])
                    if h == 0:
                        nc.vector.tensor_mul(attn[0:64, qlo:qlo + 512],
                                             av[0:64, :], rb[0:64, :])
                    else:
                        a1 = nrm.tile([64, 512], BF16, tag="a1")
                        nc.vector.tensor_mul(a1[0:64, :], av[0:64, :],
                                             rb[0:64, :])
                        nc.gpsimd.dma_start(attn[64:128, qlo:qlo + 512],
                                            a1[0:64, :])

            # ---------------- Phase C: output projection (W_o row block)
            for tp in range(8):
                o_sb = osb.tile([128, 2, D], BF16, tag="osb")
                for s in range(2):
                    tt = tp * 2 + s
                    for h5 in range(2):
                        pso = ps_mix.tile([128, 512], F32, tag="mix")
                        nc.tensor.matmul(pso[:],
                                         attn[:, tt * 128:(tt + 1) * 128],
                                         w_o[:, h5 * 512:(h5 + 1) * 512],
                                         start=True, stop=True)
                        dst = o_sb[:, s, h5 * 512:(h5 + 1) * 512]
                        if (s + h5) % 2 == 0:
                            nc.vector.tensor_copy(dst, pso[:])
                        else:
                            nc.scalar.activation(dst, pso[:], AF.Copy)
                row = b * T + tp * 256
                nc.scalar.dma_start(
                    out[row:row + 256, :].rearrange("(s p) f -> p s f", p=128),
                    o_sb[:])

    nc.compile()
    return nc


def _host_prep(x, W_qkv, W_o, token_positions):
    import ml_dtypes
    bf16 = ml_dtypes.bfloat16
    x = np.asarray(x, np.float32)
    W_qkv = np.asarray(W_qkv, np.float32)
    W_o = np.asarray(W_o, np.float32)
    pos = np.asarray(token_positions, np.float64)
    xT = np.ascontiguousarray(x.reshape(NT, D).T.astype(bf16))
    i = np.arange(32)
    inv = 1.0 / (ROPE_THETA ** (2 * i / DK))
    ang = pos[None, :] * inv[:, None]
    cos, sin = np.cos(ang), np.sin(ang)
    CC = np.tile(cos, (4, 1)).astype(np.float32)
    SS = np.concatenate([sin, -sin, sin, -sin], 0).astype(np.float32)
    trib = np.where(np.arange(128)[:, None] <= np.arange(128)[None, :],
                    0.0, -1e30).astype(bf16)
    idb = np.eye(128).astype(bf16)
    in_maps = []
    for c in range(NCORES):
        qcols, vcols = [], []
        for h in range(HPC):
            hh = HPC * c + h
            for half in range(2):
                qcols.extend(hh * DK + 2 * ii + half for ii in range(32))
            vcols.extend(hh * DK + d for d in range(DK))
        qcols = np.array(qcols)
        vcols = np.array(vcols)
        in_maps.append({
            "xT": xT,
            "wq": np.ascontiguousarray(W_qkv[:, 0 * D + qcols].astype(bf16)),
            "wk": np.ascontiguousarray((W_qkv[:, 1 * D + qcols] / 8.0).astype(bf16)),
            "wv": np.ascontiguousarray(W_qkv[:, 2 * D + vcols].astype(bf16)),
            "wo": np.ascontiguousarray(W_o[vcols, :].astype(bf16)),
            "cc": CC, "ss": SS, "trib": trib, "idb": idb,
            "ones": np.ones((128, 32), bf16),
        })
    return in_maps


def kernel(x, W_qkv, W_o, token_positions, _trace=False):
    in_maps = _host_prep(x, W_qkv, W_o, token_positions)
    if "nc" not in _BUILT:
        _BUILT["nc"] = _build_nc()
    res = run_bass_kernel_spmd(_BUILT["nc"], in_maps,
                               core_ids=list(range(NCORES)), trace=_trace)
    _BUILT["last_result"] = res
    total = np.zeros((NT, D), np.float32)
    for r in res.results:
        total += np.asarray(r["out"], dtype=np.float32)
    return total.reshape(B, T, D)


# revision 20
# speedup vs baseline: 1.2337x; 1.2240x over previous
"""Causal multi-head self-attention with RoPE on 8 Trainium2 NeuronCores.

Tensor-parallel over heads: each core handles 2 of 16 heads end-to-end
(QKV projection, RoPE, causal softmax attention, output projection with its
W_o row block). Host sums the 8 rank-128 partial outputs.

Device layouts (per core, per batch b):
  Q^T/K^T [128, 2048] bf16: rows = [h0:(even d | odd d), h1:(even d | odd d)]
  V       [128, 16, 2, 65] bf16: [t-part, t-block, head, (64 d | ones)]
  S^T     per (kb, q-chunk): rows k, cols q (causal: q >= kb*128); exp on ACT
  out^T accum via [V|1] lhsT -> row 64 = softmax denominators
All matmuls in bf16 (fp32 accumulate in PSUM); FWL fast weight loads apply.
"""
import numpy as np
from contextlib import ExitStack

import concourse.bass as bass
import concourse.tile as tile
from concourse import bacc, mybir
from concourse.bass_utils import run_bass_kernel_spmd

F32 = mybir.dt.float32
BF16 = mybir.dt.bfloat16
AF = mybir.ActivationFunctionType

D, H, DK, T, B = 1024, 16, 64, 2048, 4
NCORES, HPC = 8, 2
NT = B * T
ROPE_THETA = 10000.0
_BUILT = {}


def _build_nc():
    nc = bacc.Bacc("TRN2", target_bir_lowering=False, debug=False,
                   num_devices=NCORES)
    xT = nc.dram_tensor("xT", [D, NT], BF16, kind="ExternalInput").ap()
    wq = nc.dram_tensor("wq", [D, 128], BF16, kind="ExternalInput").ap()
    wk = nc.dram_tensor("wk", [D, 128], BF16, kind="ExternalInput").ap()
    wv = nc.dram_tensor("wv", [D, 128], BF16, kind="ExternalInput").ap()
    wo = nc.dram_tensor("wo", [128, D], BF16, kind="ExternalInput").ap()
    cc = nc.dram_tensor("cc", [128, T], F32, kind="ExternalInput").ap()
    ss = nc.dram_tensor("ss", [128, T], F32, kind="ExternalInput").ap()
    trib = nc.dram_tensor("trib", [128, 128], BF16, kind="ExternalInput").ap()
    idb = nc.dram_tensor("idb", [128, 128], BF16, kind="ExternalInput").ap()
    out = nc.dram_tensor("out", [NT, D], BF16, kind="ExternalOutput").ap()

    x3 = xT.rearrange("(dt p) n -> p dt n", p=128)   # [128, 8, NT]
    wq3 = wq.rearrange("(dt p) m -> p dt m", p=128)  # [128, 8, 128]
    wk3 = wk.rearrange("(dt p) m -> p dt m", p=128)
    wv3 = wv.rearrange("(dt p) m -> p dt m", p=128)

    with tile.TileContext(nc) as tc, ExitStack() as ctx:
        consts = ctx.enter_context(tc.tile_pool(name="consts", bufs=1))
        xin = ctx.enter_context(tc.tile_pool(name="xin", bufs=2))
        qkv = ctx.enter_context(tc.tile_pool(name="qkv", bufs=2))
        rope = ctx.enter_context(tc.tile_pool(name="rope", bufs=2))
        vtp = ctx.enter_context(tc.tile_pool(name="vtp", bufs=2))
        ptp = ctx.enter_context(tc.tile_pool(name="ptp", bufs=3))
        nrm = ctx.enter_context(tc.tile_pool(name="nrm", bufs=2))
        osb = ctx.enter_context(tc.tile_pool(name="osb", bufs=2))
        ps_s = ctx.enter_context(tc.tile_pool(name="ps_s", bufs=2, space="PSUM"))
        ps_av = ctx.enter_context(tc.tile_pool(name="ps_av", bufs=2, space="PSUM"))
        ps_mix = ctx.enter_context(tc.tile_pool(name="ps_mix", bufs=2, space="PSUM"))

        w_q = consts.tile([128, 8, 128], BF16)
        w_k = consts.tile([128, 8, 128], BF16)
        w_v = consts.tile([128, 8, 128], BF16)
        w_o = consts.tile([128, D], BF16)
        c_cc = consts.tile([128, T], F32)
        c_ss = consts.tile([128, T], F32)
        c_trib = consts.tile([128, 128], BF16)
        c_idb = consts.tile([128, 128], BF16)
        # weights on sync (ahead of x); big cos/sin tables on gpsimd queue
        # so they don't delay the first x chunk.
        nc.sync.dma_start(w_q[:], wq3)
        nc.sync.dma_start(w_k[:], wk3)
        nc.sync.dma_start(w_v[:], wv3)
        nc.sync.dma_start(w_o[:], wo)
        nc.gpsimd.dma_start(c_cc[:], cc)
        nc.gpsimd.dma_start(c_ss[:], ss)
        nc.gpsimd.dma_start(c_trib[:], trib)
        nc.gpsimd.dma_start(c_idb[:], idb)

        for b in range(B):
            # ---------------- Phase A: QKV^T projection + RoPE + V transpose
            xt = xin.tile([128, 8, T], BF16, tag="xt")
            for xc in range(4):
                nc.sync.dma_start(xt[:, :, xc * 512:(xc + 1) * 512],
                                  x3[:, :, b * T + xc * 512:b * T + (xc + 1) * 512])
            qt = qkv.tile([128, T], BF16, tag="qt")
            kt = qkv.tile([128, T], BF16, tag="kt")
            vsb = qkv.tile([128, 16, 2, 65], BF16, tag="vsb")
            nc.gpsimd.memset(vsb[:, :, :, 64:65], 1.0)
            for half in range(2):
                hlo = half * 1024
                bq = rope.tile([128, 1024], BF16, tag="bq")
                bk = rope.tile([128, 1024], BF16, tag="bk")
                bsq = rope.tile([128, 1024], BF16, tag="bsq")
                bsk = rope.tile([128, 1024], BF16, tag="bsk")
                for sub in range(2):
                    lt = hlo + sub * 512
                    ls = sub * 512
                    for which, w_sb, dest, bdest in (
                            ("q", w_q, qt, bq), ("k", w_k, kt, bk),
                            ("v", w_v, None, None)):
                        psA = ps_mix.tile([128, 512], F32, tag="mix")
                        for dt_i in range(8):
                            nc.tensor.matmul(psA[:], w_sb[:, dt_i, :],
                                             xt[:, dt_i, lt:lt + 512],
                                             start=(dt_i == 0), stop=(dt_i == 7))
                        if which == "v":
                            vt = vtp.tile([128, 512], BF16, tag="vt")
                            nc.vector.tensor_copy(vt[:], psA[:])
                            for s in range(4):
                                g = (lt // 128) + s
                                ptr = ps_mix.tile([128, 128], BF16, tag="mix")
                                nc.tensor.transpose(
                                    ptr[:], vt[:, s * 128:(s + 1) * 128],
                                    c_idb[:])
                                dst = vsb[:, g, :, 0:64]
                                src = ptr[:].rearrange("p (h d) -> p h d", h=2)
                                nc.scalar.activation(dst, src, AF.Copy)
                        else:
                            nc.vector.tensor_mul(dest[:, lt:lt + 512], psA[:],
                                                 c_cc[:, lt:lt + 512])
                            nc.vector.tensor_mul(bdest[:, ls:ls + 512], psA[:],
                                                 c_ss[:, lt:lt + 512])
                # RoPE cross-term: swap 32-row halves within each head block,
                # add into q^T/k^T. Swap DMAs go on the gpsimd queue so they
                # don't sit behind x loads (sync) or exps (scalar).
                for bt, bst, dest in ((bq, bsq, qt), (bk, bsk, kt)):
                    for hh in range(2):
                        r0 = hh * 64
                        nc.gpsimd.dma_start(bst[r0 + 32:r0 + 64, :],
                                            bt[r0:r0 + 32, :])
                        nc.gpsimd.dma_start(bst[r0:r0 + 32, :],
                                            bt[r0 + 32:r0 + 64, :])
                    nc.vector.tensor_add(dest[:, hlo:hlo + 1024],
                                         dest[:, hlo:hlo + 1024], bst[:])

            # ---------------- Phase B: causal attention, heads packed on
            # PE row-groups (h0 rows 0-63, h1 rows 64-127)
            attn = qkv.tile([128, T], BF16, tag="attnT")
            for qc in range(4):
                av0 = ps_av.tile([65, 512], F32, tag="av")
                av1 = ps_av.tile([65, 512], F32, tag="av")
                nkb = 4 * qc + 4
                pend = None  # software pipeline: AV trails S/exp by one kb
                for kb in range(nkb):
                    k0 = kb * 128
                    q0 = max(qc * 512, k0)
                    q1 = qc * 512 + 512
                    n = q1 - q0
                    diag = (q0 == k0)
                    sps = ps_s.tile([128, 1024], F32, tag="sps")
                    for h in range(2):
                        nc.tensor.matmul(
                            sps[:, h * 512:h * 512 + n],
                            kt[h * 64:(h + 1) * 64, k0:k0 + 128],
                            qt[h * 64:(h + 1) * 64, q0:q1],
                            start=True, stop=not diag,
                            skip_group_check=True)
                    if diag:  # causal mask: -1e30 upper triangle via PE
                        for h in range(2):
                            nc.tensor.matmul(
                                sps[:, h * 512:h * 512 + 128],
                                c_idb[:], c_trib[:],
                                start=False, stop=True,
                                skip_group_check=True)
                    pt = ptp.tile([128, 2, 512], BF16, tag="pt")
                    nc.scalar.activation(
                        pt[:, :, 0:n],
                        sps[:].rearrange("p (h f) -> p h f", h=2)[:, :, 0:n],
                        AF.Exp)
                    if pend is not None:
                        p_pt, p_q0, p_n, p_kb = pend
                        for h, av in ((0, av0), (1, av1)):
                            nc.tensor.matmul(
                                av[:, p_q0 - qc * 512:512],
                                vsb[:, p_kb, h, :], p_pt[:, h, 0:p_n],
                                start=(p_kb == 0), stop=False,
                                skip_group_check=True)
                    pend = (pt, q0, n, kb)
                p_pt, p_q0, p_n, p_kb = pend
                for h, av in ((0, av0), (1, av1)):
                    nc.tensor.matmul(
                        av[:, p_q0 - qc * 512:512],
                        vsb[:, p_kb, h, :], p_pt[:, h, 0:p_n],
                        start=(p_kb == 0), stop=True,
                        skip_group_check=True)
                # normalize both heads; fast approx reciprocal (err ~51 ULP,
                # plenty for softmax denominators), broadcast on gpsimd
                qlo = qc * 512
                for h, av in ((0, av0), (1, av1)):
                    # denom row -> [128,4] via DMA so the exact reciprocal
                    # runs on 128 partitions (cost ~ cols), then back to a
                    # [1,512] row for the partition broadcast.
                    rcps = nrm.tile([128, 2, 512], F32, tag="rcps")
                    rsq = nrm.tile([128, 8], F32, tag="rsq")
                    nc.vector.tensor_copy(rcps[64:65, 0, :], av[64:65, :])
                    nc.gpsimd.dma_start(rsq[:, 0:4], rcps[64:65, 0, :])
                    nc.vector.reciprocal(rsq[:, 4:8], rsq[:, 0:4])
                    nc.gpsimd.dma_start(rcps[0:1, 1, :], rsq[:, 4:8])
                    rb = nrm.tile([64, 512], F32, tag="rb")
                    nc.gpsimd.partition_broadcast(rb[0:64, :], rcps[0:1, 1, :])
                    if h == 0:
                        nc.vector.tensor_mul(attn[0:64, qlo:qlo + 512],
                                             av[0:64, :], rb[0:64, :])
                    else:
                        a1 = nrm.tile([64, 512], BF16, tag="a1")
                        nc.vector.tensor_mul(a1[0:64, :], av[0:64, :],
                                             rb[0:64, :])
                        nc.gpsimd.dma_start(attn[64:128, qlo:qlo + 512],
                                            a1[0:64, :])

            # ---------------- Phase C: output projection (W_o row block)
            for tp in range(8):
                o_sb = osb.tile([128, 2, D], BF16, tag="osb")
                for s in range(2):
                    tt = tp * 2 + s
                    for h5 in range(2):
                        # pso lives in the "sps" slots: phase C runs after
                        # this batch's phase B, so sps banks are free and
                        # next-batch phase A keeps its own "mix" slots.
                        pso = ps_s.tile([128, 512], F32, tag="sps")
                        nc.tensor.matmul(pso[:],
                                         attn[:, tt * 128:(tt + 1) * 128],
                                         w_o[:, h5 * 512:(h5 + 1) * 512],
                                         start=True, stop=True)
                        dst = o_sb[:, s, h5 * 512:(h5 + 1) * 512]
                        nc.vector.tensor_copy(dst, pso[:])
                row = b * T + tp * 256
                nc.scalar.dma_start(
                    out[row:row + 256, :].rearrange("(s p) f -> p s f", p=128),
                    o_sb[:])

    nc.compile()
    return nc


def _host_prep(x, W_qkv, W_o, token_positions):
    import ml_dtypes
    bf16 = ml_dtypes.bfloat16
    x = np.asarray(x, np.float32)
    W_qkv = np.asarray(W_qkv, np.float32)
    W_o = np.asarray(W_o, np.float32)
    pos = np.asarray(token_positions, np.float64)
    xT = np.ascontiguousarray(x.reshape(NT, D).T.astype(bf16))
    i = np.arange(32)
    inv = 1.0 / (ROPE_THETA ** (2 * i / DK))
    ang = pos[None, :] * inv[:, None]
    cos, sin = np.cos(ang), np.sin(ang)
    CC = np.tile(cos, (4, 1)).astype(np.float32)
    SS = np.concatenate([sin, -sin, sin, -sin], 0).astype(np.float32)
    trib = np.where(np.arange(128)[:, None] <= np.arange(128)[None, :],
                    0.0, -1e30).astype(bf16)
    idb = np.eye(128).astype(bf16)
    in_maps = []
    for c in range(NCORES):
        qcols, vcols = [], []
        for h in range(HPC):
            hh = HPC * c + h
            for half in range(2):
                qcols.extend(hh * DK + 2 * ii + half for ii in range(32))
            vcols.extend(hh * DK + d for d in range(DK))
        qcols = np.array(qcols)
        vcols = np.array(vcols)
        in_maps.append({
            "xT": xT,
            "wq": np.ascontiguousarray(W_qkv[:, 0 * D + qcols].astype(bf16)),
            "wk": np.ascontiguousarray((W_qkv[:, 1 * D + qcols] / 8.0).astype(bf16)),
            "wv": np.ascontiguousarray(W_qkv[:, 2 * D + vcols].astype(bf16)),
            "wo": np.ascontiguousarray(W_o[vcols, :].astype(bf16)),
            "cc": CC, "ss": SS, "trib": trib, "idb": idb,
        })
    return in_maps


def kernel(x, W_qkv, W_o, token_positions, _trace=False):
    in_maps = _host_prep(x, W_qkv, W_o, token_positions)
    if "nc" not in _BUILT:
        _BUILT["nc"] = _build_nc()
    res = run_bass_kernel_spmd(_BUILT["nc"], in_maps,
                               core_ids=list(range(NCORES)), trace=_trace)
    _BUILT["last_result"] = res
    total = np.zeros((NT, D), np.float32)
    for r in res.results:
        total += np.asarray(r["out"], dtype=np.float32)
    return total.reshape(B, T, D)


# revision 54
# speedup vs baseline: 1.5537x; 1.2593x over previous
"""Causal multi-head self-attention with RoPE on 8 Trainium2 NeuronCores.

Tensor-parallel over heads: each core handles 2 of 16 heads end-to-end
(QKV projection, RoPE, causal softmax attention, output projection with its
W_o row block). Host sums the 8 rank-128 partial outputs.

Device layouts (per core, per batch b):
  Q^T/K^T [128, 2048] bf16: rows = [h0:(even d | odd d), h1:(even d | odd d)]
  V       [128, 16, 2, 65] bf16: [t-part, t-block, head, (64 d | ones)]
  S^T     per (kb, q-chunk): rows k, cols q (causal: q >= kb*128); exp on ACT
  out^T accum via [V|1] lhsT -> row 64 = softmax denominators
All matmuls in bf16 (fp32 accumulate in PSUM); FWL fast weight loads apply.
"""
import numpy as np
from contextlib import ExitStack

import concourse.bass as bass
import concourse.tile as tile
from concourse import bacc, mybir
from concourse.bass_utils import run_bass_kernel_spmd

F32 = mybir.dt.float32
BF16 = mybir.dt.bfloat16
AF = mybir.ActivationFunctionType

D, H, DK, T, B = 1024, 16, 64, 2048, 4
NCORES, HPC = 8, 2
NT = B * T
ROPE_THETA = 10000.0
_BUILT = {}


def _build_nc():
    nc = bacc.Bacc("TRN2", target_bir_lowering=False, debug=False,
                   num_devices=NCORES)
    xT = nc.dram_tensor("xT", [D, NT], BF16, kind="ExternalInput").ap()
    wq = nc.dram_tensor("wq", [D, 128], BF16, kind="ExternalInput").ap()
    wk = nc.dram_tensor("wk", [D, 128], BF16, kind="ExternalInput").ap()
    wv = nc.dram_tensor("wv", [D, 128], BF16, kind="ExternalInput").ap()
    wo = nc.dram_tensor("wo", [128, D], BF16, kind="ExternalInput").ap()
    cc = nc.dram_tensor("cc", [128, T], F32, kind="ExternalInput").ap()
    ss = nc.dram_tensor("ss", [128, T], F32, kind="ExternalInput").ap()
    trib = nc.dram_tensor("trib", [128, 128], BF16, kind="ExternalInput").ap()
    idb = nc.dram_tensor("idb", [128, 128], BF16, kind="ExternalInput").ap()
    out = nc.dram_tensor("out", [NT, D], BF16, kind="ExternalOutput").ap()

    x3 = xT.rearrange("(dt p) n -> p dt n", p=128)   # [128, 8, NT]
    wq3 = wq.rearrange("(dt p) m -> p dt m", p=128)  # [128, 8, 128]
    wk3 = wk.rearrange("(dt p) m -> p dt m", p=128)
    wv3 = wv.rearrange("(dt p) m -> p dt m", p=128)

    with tile.TileContext(nc) as tc, ExitStack() as ctx:
        consts = ctx.enter_context(tc.tile_pool(name="consts", bufs=1))
        xin = ctx.enter_context(tc.tile_pool(name="xin", bufs=2))
        qkv = ctx.enter_context(tc.tile_pool(name="qkv", bufs=2))
        rope = ctx.enter_context(tc.tile_pool(name="rope", bufs=2))
        vtp = ctx.enter_context(tc.tile_pool(name="vtp", bufs=3))
        ptp = ctx.enter_context(tc.tile_pool(name="ptp", bufs=4))
        nrm = ctx.enter_context(tc.tile_pool(name="nrm", bufs=2))
        osb = ctx.enter_context(tc.tile_pool(name="osb", bufs=2))
        ps_s = ctx.enter_context(tc.tile_pool(name="ps_s", bufs=2, space="PSUM"))
        ps_av = ctx.enter_context(tc.tile_pool(name="ps_av", bufs=2, space="PSUM"))
        ps_mix = ctx.enter_context(tc.tile_pool(name="ps_mix", bufs=2, space="PSUM"))

        w_q = consts.tile([128, 8, 128], BF16)
        w_k = consts.tile([128, 8, 128], BF16)
        w_v = consts.tile([128, 8, 128], BF16)
        w_o = consts.tile([128, D], BF16)
        c_cc = consts.tile([128, T], F32)
        c_ss = consts.tile([128, T], F32)
        c_tri01 = consts.tile([128, 128], BF16)
        c_idb = consts.tile([128, 128], BF16)
        # weights on sync (ahead of x); big cos/sin tables on gpsimd queue
        # so they don't delay the first x chunk.
        nc.sync.dma_start(w_q[:], wq3)
        nc.sync.dma_start(w_k[:], wk3)
        nc.sync.dma_start(w_v[:], wv3)
        nc.sync.dma_start(w_o[:], wo)
        nc.gpsimd.dma_start(c_cc[:], cc)
        nc.gpsimd.dma_start(c_ss[:], ss)
        nc.gpsimd.dma_start(c_tri01[:], trib)
        nc.gpsimd.dma_start(c_idb[:], idb)

        # -- per-batch tile sets; phase A of batch b+1 is EMITTED interleaved
        # with phase B of batch b so the PE has projection matmuls to chew on
        # while ACT computes exps (otherwise it stalls at every qc boundary).
        def alloc_batch(b):
            xt = xin.tile([128, 8, T], BF16, tag="xt")
            for xc in range(4):
                nc.sync.dma_start(
                    xt[:, :, xc * 512:(xc + 1) * 512],
                    x3[:, :, b * T + xc * 512:b * T + (xc + 1) * 512])
            qt = qkv.tile([128, T], BF16, tag="qt")
            kt = qkv.tile([128, T], BF16, tag="kt")
            vsb = qkv.tile([128, 16, 2, 65], BF16, tag="vsb")
            nc.gpsimd.memset(vsb[:, :, :, 64:65], 1.0)
            return {"xt": xt, "qt": qt, "kt": kt, "vsb": vsb, "rope": {},
                    "vt": {}}

        def emit_A_qk(tl, which, half, sub):
            # q^T or k^T projection of one 512-token chunk + RoPE muls
            if which == "q" and sub == 0:
                bq_t = rope.tile([128, 1024], BF16, tag="bq")
                bk_t = rope.tile([128, 1024], BF16, tag="bk")
                tl["rope"][half] = (bq_t, bk_t)
            bq, bk = tl["rope"][half]
            w_sb, dest, bdest = ((w_q, tl["qt"], bq) if which == "q"
                                 else (w_k, tl["kt"], bk))
            lt = half * 1024 + sub * 512
            ls = sub * 512
            psA = ps_mix.tile([128, 512], F32, tag="mix")
            for dt_i in range(8):
                nc.tensor.matmul(psA[:], w_sb[:, dt_i, :],
                                 tl["xt"][:, dt_i, lt:lt + 512],
                                 start=(dt_i == 0), stop=(dt_i == 7))
            nc.vector.tensor_mul(dest[:, lt:lt + 512], psA[:],
                                 c_cc[:, lt:lt + 512])
            nc.vector.tensor_mul(bdest[:, ls:ls + 512], psA[:],
                                 c_ss[:, lt:lt + 512])

        def emit_A_v(tl, half, sub):
            # V^T projection of one 512-token chunk, staged to SBUF bf16
            lt = half * 1024 + sub * 512
            psA = ps_mix.tile([128, 512], F32, tag="mix")
            for dt_i in range(8):
                nc.tensor.matmul(psA[:], w_v[:, dt_i, :],
                                 tl["xt"][:, dt_i, lt:lt + 512],
                                 start=(dt_i == 0), stop=(dt_i == 7))
            vt = vtp.tile([128, 512], BF16, tag="vt")
            nc.vector.tensor_copy(vt[:], psA[:])
            tl["vt"][(half, sub)] = vt

        def emit_A_transp(tl, half, sub):
            # deferred V transposes (vt copy long done): 4 PE transposes
            # into one PSUM bank, single strided drain into vsb
            lt = half * 1024 + sub * 512
            vt = tl["vt"].pop((half, sub))
            g0 = lt // 128
            ptr = ps_mix.tile([128, 4, 128], BF16, tag="mix")
            for s in range(4):
                nc.tensor.transpose(
                    ptr[:, s, :], vt[:, s * 128:(s + 1) * 128], c_idb[:])
            dst = tl["vsb"][:, g0:g0 + 4, :, 0:64]
            src = ptr[:].rearrange("p s (h d) -> p s h d", h=2)
            nc.vector.tensor_copy(dst, src)

        def emit_rope_tail(tl, half):
            # RoPE cross-term: swap 32-row halves within each head block,
            # add into q^T/k^T. Swap DMAs on the gpsimd queue.
            hlo = half * 1024
            bq, bk = tl["rope"][half]
            bsq = rope.tile([128, 1024], BF16, tag="bsq")
            bsk = rope.tile([128, 1024], BF16, tag="bsk")
            for bt, bst, dest in ((bq, bsq, tl["qt"]), (bk, bsk, tl["kt"])):
                for hh in range(2):
                    r0 = hh * 64
                    nc.gpsimd.dma_start(bst[r0 + 32:r0 + 64, :],
                                        bt[r0:r0 + 32, :])
                    nc.gpsimd.dma_start(bst[r0:r0 + 32, :],
                                        bt[r0 + 32:r0 + 64, :])
                nc.vector.tensor_add(dest[:, hlo:hlo + 1024],
                                     dest[:, hlo:hlo + 1024], bst[:])

        def a_items(tl):
            # phase A as a list of small PE work items (~1-2us each) to be
            # drip-fed between phase-B kb iterations of the previous batch.
            # Each chunk's V transposes are deferred into the next chunk so
            # their LDW never waits on the vt drain.
            items = []
            prev = None
            for step in range(4):
                half, sub = step // 2, step % 2
                items.append(lambda tl=tl, h=half, s=sub:
                             emit_A_qk(tl, "q", h, s))
                if prev is not None:
                    items.append(lambda tl=tl, p=prev:
                                 emit_A_transp(tl, p[0], p[1]))
                items.append(lambda tl=tl, h=half, s=sub:
                             emit_A_qk(tl, "k", h, s))
                items.append(lambda tl=tl, h=half, s=sub:
                             emit_A_v(tl, h, s))
                if sub == 1:
                    items.append(lambda tl=tl, h=half:
                                 emit_rope_tail(tl, h))
                prev = (half, sub)
            items.append(lambda tl=tl, p=prev:
                         emit_A_transp(tl, p[0], p[1]))
            return items

        def emit_B_qc(tl, qc, filler):
            qt, kt, vsb, attn = tl["qt"], tl["kt"], tl["vsb"], tl["attn"]
            av0 = ps_av.tile([65, 512], F32, tag="av")
            av1 = ps_av.tile([65, 512], F32, tag="av")
            nkb = 4 * qc + 4
            pend = None  # software pipeline: AV trails S/exp by one kb

            def emit_av(p, last):
                p_pt, p_q0, p_n, p_kb = p
                for h, av in ((0, av0), (1, av1)):
                    nc.tensor.matmul(
                        av[:, p_q0 - qc * 512:512],
                        vsb[:, p_kb, h, :], p_pt[:, h, 0:p_n],
                        start=(p_kb == 0), stop=last,
                        skip_group_check=True)

            for kb in range(nkb):
                k0 = kb * 128
                q0 = max(qc * 512, k0)
                q1 = qc * 512 + 512
                n = q1 - q0
                diag = (q0 == k0)
                sps = ps_s.tile([128, 1024], F32, tag="sps")
                for h in range(2):
                    nc.tensor.matmul(
                        sps[:, h * 512:h * 512 + n],
                        kt[h * 64:(h + 1) * 64, k0:k0 + 128],
                        qt[h * 64:(h + 1) * 64, q0:q1],
                        start=True, stop=True,
                        skip_group_check=True)
                pt = ptp.tile([128, 2, 512], BF16, tag="pt")
                nc.scalar.activation(
                    pt[:, :, 0:n],
                    sps[:].rearrange("p (h f) -> p h f", h=2)[:, :, 0:n],
                    AF.Exp)
                if diag:  # causal mask: zero the upper triangle post-exp
                    for h in range(2):
                        nc.vector.tensor_mul(pt[:, h, 0:128], pt[:, h, 0:128],
                                             c_tri01[:])
                if pend is not None:
                    emit_av(pend, last=False)
                pend = (pt, q0, n, kb)
                filler()  # drip one next-batch phase-A item into the gap
            emit_av(pend, last=True)

            # normalize both heads: denom row -> [128,4] via DMA so the exact
            # reciprocal runs across partitions (cost ~ cols), back to a
            # [1,512] row, broadcast on gpsimd, scale on DVE.
            qlo = qc * 512
            for h, av in ((0, av0), (1, av1)):
                rcps = nrm.tile([128, 2, 512], F32, tag="rcps")
                rsq = nrm.tile([128, 8], F32, tag="rsq")
                nc.vector.tensor_copy(rcps[64:65, 0, :], av[64:65, :])
                nc.gpsimd.dma_start(rsq[:, 0:4], rcps[64:65, 0, :])
                nc.vector.reciprocal(rsq[:, 4:8], rsq[:, 0:4])
                nc.gpsimd.dma_start(rcps[0:1, 1, :], rsq[:, 4:8])
                rb = nrm.tile([64, 512], F32, tag="rb")
                nc.gpsimd.partition_broadcast(rb[0:64, :], rcps[0:1, 1, :])
                if h == 0:
                    nc.vector.tensor_mul(attn[0:64, qlo:qlo + 512],
                                         av[0:64, :], rb[0:64, :])
                else:
                    a1 = nrm.tile([64, 512], BF16, tag="a1")
                    nc.vector.tensor_mul(a1[0:64, :], av[0:64, :],
                                         rb[0:64, :])
                    # scalar queue: phase-C's first LDW waits on this move,
                    # and the gpsimd queue is busy with swaps/normalize DMAs
                    nc.scalar.dma_start(attn[64:128, qlo:qlo + 512],
                                        a1[0:64, :])

        def emit_C(tl, b):
            # output projection: both h5 halves into one 2-bank pso, single
            # DVE drain per (tp, s) so phase C isn't drain-serialized.
            attn = tl["attn"]
            for tp in range(8):
                o_sb = osb.tile([128, 2, D], BF16, tag="osb")
                for s in range(2):
                    tt = tp * 2 + s
                    pso = ps_s.tile([128, 2, 512], F32, tag="sps")
                    for h5 in range(2):
                        nc.tensor.matmul(pso[:, h5, :],
                                         attn[:, tt * 128:(tt + 1) * 128],
                                         w_o[:, h5 * 512:(h5 + 1) * 512],
                                         start=True, stop=True)
                    # alternate drain engine: ACT is idle during phase C and
                    # the DVE drain stream otherwise gates the slot rotation
                    src = pso[:].rearrange("p h f -> p (h f)")
                    if s == 0:
                        nc.vector.tensor_copy(o_sb[:, s, :], src)
                    else:
                        nc.scalar.activation(o_sb[:, s, :], src, AF.Copy)
                row = b * T + tp * 256
                nc.scalar.dma_start(
                    out[row:row + 256, :].rearrange("(s p) f -> p s f", p=128),
                    o_sb[:])

        tiles = {0: alloc_batch(0)}
        for item in a_items(tiles[0]):
            item()
        for b in range(B):
            tl = tiles[b]
            attn_t = qkv.tile([128, T], BF16, tag="attnT")
            tl["attn"] = attn_t
            queue = []
            if b + 1 < B:
                tiles[b + 1] = alloc_batch(b + 1)
                queue = a_items(tiles[b + 1])
            qi = [0]

            def filler():
                if qi[0] < len(queue):
                    queue[qi[0]]()
                    qi[0] += 1

            for qc in range(4):
                emit_B_qc(tl, qc, filler)
            while qi[0] < len(queue):
                queue[qi[0]]()
                qi[0] += 1
            emit_C(tl, b)
            del tiles[b]

    nc.compile()
    return nc


def _host_prep(x, W_qkv, W_o, token_positions):
    import ml_dtypes
    bf16 = ml_dtypes.bfloat16
    x = np.asarray(x, np.float32)
    W_qkv = np.asarray(W_qkv, np.float32)
    W_o = np.asarray(W_o, np.float32)
    pos = np.asarray(token_positions, np.float64)
    xT = np.ascontiguousarray(x.reshape(NT, D).T.astype(bf16))
    i = np.arange(32)
    inv = 1.0 / (ROPE_THETA ** (2 * i / DK))
    ang = pos[None, :] * inv[:, None]
    cos, sin = np.cos(ang), np.sin(ang)
    CC = np.tile(cos, (4, 1)).astype(np.float32)
    SS = np.concatenate([sin, -sin, sin, -sin], 0).astype(np.float32)
    trib = np.where(np.arange(128)[:, None] <= np.arange(128)[None, :],
                    1.0, 0.0).astype(bf16)
    idb = np.eye(128).astype(bf16)
    in_maps = []
    for c in range(NCORES):
        qcols, vcols = [], []
        for h in range(HPC):
            hh = HPC * c + h
            for half in range(2):
                qcols.extend(hh * DK + 2 * ii + half for ii in range(32))
            vcols.extend(hh * DK + d for d in range(DK))
        qcols = np.array(qcols)
        vcols = np.array(vcols)
        in_maps.append({
            "xT": xT,
            "wq": np.ascontiguousarray(W_qkv[:, 0 * D + qcols].astype(bf16)),
            "wk": np.ascontiguousarray((W_qkv[:, 1 * D + qcols] / 8.0).astype(bf16)),
            "wv": np.ascontiguousarray(W_qkv[:, 2 * D + vcols].astype(bf16)),
            "wo": np.ascontiguousarray(W_o[vcols, :].astype(bf16)),
            "cc": CC, "ss": SS, "trib": trib, "idb": idb,
        })
    return in_maps


def kernel(x, W_qkv, W_o, token_positions, _trace=False):
    in_maps = _host_prep(x, W_qkv, W_o, token_positions)
    if "nc" not in _BUILT:
        _BUILT["nc"] = _build_nc()
    res = run_bass_kernel_spmd(_BUILT["nc"], in_maps,
                               core_ids=list(range(NCORES)), trace=_trace)
    _BUILT["last_result"] = res
    total = np.zeros((NT, D), np.float32)
    for r in res.results:
        total += np.asarray(r["out"], dtype=np.float32)
    return total.reshape(B, T, D)


# revision 61
# speedup vs baseline: 1.6027x; 1.0315x over previous
"""Causal multi-head self-attention with RoPE on 8 Trainium2 NeuronCores.

Tensor-parallel over heads: each core handles 2 of 16 heads end-to-end
(QKV projection, RoPE, causal softmax attention, output projection with its
W_o row block). Host sums the 8 rank-128 partial outputs.

Device layouts (per core, per batch b):
  Q^T/K^T [128, 2048] bf16: rows = [h0:(even d | odd d), h1:(even d | odd d)]
  V       [128, 16, 2, 65] bf16: [t-part, t-block, head, (64 d | ones)]
  S^T     per (kb, q-chunk): rows k, cols q (causal: q >= kb*128); exp on ACT
  out^T accum via [V|1] lhsT -> row 64 = softmax denominators
All matmuls in bf16 (fp32 accumulate in PSUM); FWL fast weight loads apply.
"""
import numpy as np
from contextlib import ExitStack

import concourse.bass as bass
import concourse.tile as tile
from concourse import bacc, mybir
from concourse.bass_utils import run_bass_kernel_spmd

F32 = mybir.dt.float32
BF16 = mybir.dt.bfloat16
AF = mybir.ActivationFunctionType

D, H, DK, T, B = 1024, 16, 64, 2048, 4
NCORES, HPC = 8, 2
NT = B * T
ROPE_THETA = 10000.0
_BUILT = {}


def _build_nc():
    nc = bacc.Bacc("TRN2", target_bir_lowering=False, debug=False,
                   num_devices=NCORES)
    xT = nc.dram_tensor("xT", [D, NT], BF16, kind="ExternalInput").ap()
    wq = nc.dram_tensor("wq", [D, 128], BF16, kind="ExternalInput").ap()
    wk = nc.dram_tensor("wk", [D, 128], BF16, kind="ExternalInput").ap()
    wv = nc.dram_tensor("wv", [D, 128], BF16, kind="ExternalInput").ap()
    wo = nc.dram_tensor("wo", [128, D], BF16, kind="ExternalInput").ap()
    cc = nc.dram_tensor("cc", [128, T], F32, kind="ExternalInput").ap()
    ss = nc.dram_tensor("ss", [128, T], F32, kind="ExternalInput").ap()
    trib = nc.dram_tensor("trib", [128, 128], BF16, kind="ExternalInput").ap()
    idb = nc.dram_tensor("idb", [128, 128], BF16, kind="ExternalInput").ap()
    out = nc.dram_tensor("out", [NT, D], BF16, kind="ExternalOutput").ap()

    x3 = xT.rearrange("(dt p) n -> p dt n", p=128)   # [128, 8, NT]
    wq3 = wq.rearrange("(dt p) m -> p dt m", p=128)  # [128, 8, 128]
    wk3 = wk.rearrange("(dt p) m -> p dt m", p=128)
    wv3 = wv.rearrange("(dt p) m -> p dt m", p=128)

    with tile.TileContext(nc) as tc, ExitStack() as ctx:
        consts = ctx.enter_context(tc.tile_pool(name="consts", bufs=1))
        xin = ctx.enter_context(tc.tile_pool(name="xin", bufs=2))
        qkv = ctx.enter_context(tc.tile_pool(name="qkv", bufs=2))
        rope = ctx.enter_context(tc.tile_pool(name="rope", bufs=2))
        vtp = ctx.enter_context(tc.tile_pool(name="vtp", bufs=3))
        ptp = ctx.enter_context(tc.tile_pool(name="ptp", bufs=4))
        nrm = ctx.enter_context(tc.tile_pool(name="nrm", bufs=2))
        osb = ctx.enter_context(tc.tile_pool(name="osb", bufs=2))
        ps_s = ctx.enter_context(tc.tile_pool(name="ps_s", bufs=2, space="PSUM"))
        ps_av = ctx.enter_context(tc.tile_pool(name="ps_av", bufs=2, space="PSUM"))
        ps_mix = ctx.enter_context(tc.tile_pool(name="ps_mix", bufs=2, space="PSUM"))

        w_q = consts.tile([128, 8, 128], BF16)
        w_k = consts.tile([128, 8, 128], BF16)
        w_v = consts.tile([128, 8, 128], BF16)
        w_o = consts.tile([128, D], BF16)
        c_cc = consts.tile([128, T], F32)
        c_ss = consts.tile([128, T], F32)
        c_tri01 = consts.tile([128, 128], BF16)
        c_idb = consts.tile([128, 128], BF16)
        # weights on sync (ahead of x); big cos/sin tables on gpsimd queue
        # so they don't delay the first x chunk.
        nc.sync.dma_start(w_q[:], wq3)
        nc.sync.dma_start(w_k[:], wk3)
        nc.sync.dma_start(w_v[:], wv3)
        nc.sync.dma_start(w_o[:], wo)
        nc.gpsimd.dma_start(c_cc[:], cc)
        nc.gpsimd.dma_start(c_ss[:], ss)
        nc.gpsimd.dma_start(c_tri01[:], trib)
        nc.gpsimd.dma_start(c_idb[:], idb)

        # -- per-batch tile sets; phase A of batch b+1 is EMITTED interleaved
        # with phase B of batch b so the PE has projection matmuls to chew on
        # while ACT computes exps (otherwise it stalls at every qc boundary).
        def alloc_batch(b):
            xt = xin.tile([128, 8, T], BF16, tag="xt")
            for xc in range(4):
                nc.sync.dma_start(
                    xt[:, :, xc * 512:(xc + 1) * 512],
                    x3[:, :, b * T + xc * 512:b * T + (xc + 1) * 512])
            qt = qkv.tile([128, T], BF16, tag="qt")
            kt = qkv.tile([128, T], BF16, tag="kt")
            vsb = qkv.tile([128, 16, 2, 65], BF16, tag="vsb")
            nc.gpsimd.memset(vsb[:, :, :, 64:65], 1.0)
            return {"xt": xt, "qt": qt, "kt": kt, "vsb": vsb, "rope": {},
                    "vt": {}}

        def emit_A_qk(tl, which, half, sub):
            # q^T or k^T projection of one 512-token chunk + RoPE muls
            if which == "q" and sub == 0:
                bq_t = rope.tile([128, 1024], BF16, tag="bq")
                bk_t = rope.tile([128, 1024], BF16, tag="bk")
                tl["rope"][half] = (bq_t, bk_t)
            bq, bk = tl["rope"][half]
            w_sb, dest, bdest = ((w_q, tl["qt"], bq) if which == "q"
                                 else (w_k, tl["kt"], bk))
            lt = half * 1024 + sub * 512
            ls = sub * 512
            psA = ps_mix.tile([128, 512], F32, tag="mix")
            for dt_i in range(8):
                nc.tensor.matmul(psA[:], w_sb[:, dt_i, :],
                                 tl["xt"][:, dt_i, lt:lt + 512],
                                 start=(dt_i == 0), stop=(dt_i == 7))
            nc.vector.tensor_mul(dest[:, lt:lt + 512], psA[:],
                                 c_cc[:, lt:lt + 512])
            nc.vector.tensor_mul(bdest[:, ls:ls + 512], psA[:],
                                 c_ss[:, lt:lt + 512])

        def emit_A_v(tl, half, sub):
            # V^T projection of one 512-token chunk, staged to SBUF bf16
            lt = half * 1024 + sub * 512
            psA = ps_mix.tile([128, 512], F32, tag="mix")
            for dt_i in range(8):
                nc.tensor.matmul(psA[:], w_v[:, dt_i, :],
                                 tl["xt"][:, dt_i, lt:lt + 512],
                                 start=(dt_i == 0), stop=(dt_i == 7))
            vt = vtp.tile([128, 512], BF16, tag="vt")
            nc.vector.tensor_copy(vt[:], psA[:])
            tl["vt"][(half, sub)] = vt

        def emit_A_transp(tl, half, sub):
            # deferred V transposes (vt copy long done): 4 PE transposes
            # into one PSUM bank, single strided drain into vsb
            lt = half * 1024 + sub * 512
            vt = tl["vt"].pop((half, sub))
            g0 = lt // 128
            ptr = ps_mix.tile([128, 4, 128], BF16, tag="mix")
            for s in range(4):
                nc.tensor.transpose(
                    ptr[:, s, :], vt[:, s * 128:(s + 1) * 128], c_idb[:])
            dst = tl["vsb"][:, g0:g0 + 4, :, 0:64]
            src = ptr[:].rearrange("p s (h d) -> p s h d", h=2)
            nc.vector.tensor_copy(dst, src)

        def emit_rope_tail(tl, half):
            # RoPE cross-term: swap 32-row halves within each head block,
            # add into q^T/k^T. Swap DMAs on the gpsimd queue.
            hlo = half * 1024
            bq, bk = tl["rope"][half]
            bsq = rope.tile([128, 1024], BF16, tag="bsq")
            bsk = rope.tile([128, 1024], BF16, tag="bsk")
            for bt, bst, dest in ((bq, bsq, tl["qt"]), (bk, bsk, tl["kt"])):
                for hh in range(2):
                    r0 = hh * 64
                    nc.gpsimd.dma_start(bst[r0 + 32:r0 + 64, :],
                                        bt[r0:r0 + 32, :])
                    nc.gpsimd.dma_start(bst[r0:r0 + 32, :],
                                        bt[r0 + 32:r0 + 64, :])
                nc.vector.tensor_add(dest[:, hlo:hlo + 1024],
                                     dest[:, hlo:hlo + 1024], bst[:])

        def a_items(tl):
            # phase A as a list of small PE work items (~1-2us each) to be
            # drip-fed between phase-B kb iterations of the previous batch.
            # Each chunk's V transposes are deferred into the next chunk so
            # their LDW never waits on the vt drain.
            items = []
            prev = None
            for step in range(4):
                half, sub = step // 2, step % 2
                items.append(lambda tl=tl, h=half, s=sub:
                             emit_A_qk(tl, "q", h, s))
                if prev is not None:
                    items.append(lambda tl=tl, p=prev:
                                 emit_A_transp(tl, p[0], p[1]))
                items.append(lambda tl=tl, h=half, s=sub:
                             emit_A_qk(tl, "k", h, s))
                items.append(lambda tl=tl, h=half, s=sub:
                             emit_A_v(tl, h, s))
                if sub == 1:
                    items.append(lambda tl=tl, h=half:
                                 emit_rope_tail(tl, h))
                prev = (half, sub)
            items.append(lambda tl=tl, p=prev:
                         emit_A_transp(tl, p[0], p[1]))
            return items

        def emit_B_qc(tl, qc, filler):
            qt, kt, vsb, attn = tl["qt"], tl["kt"], tl["vsb"], tl["attn"]
            av0 = ps_av.tile([65, 512], F32, tag="av")
            av1 = ps_av.tile([65, 512], F32, tag="av")
            nkb = 4 * qc + 4
            pend = None  # software pipeline: AV trails S/exp by one kb

            def emit_av(p, last):
                p_pt, p_q0, p_n, p_kb = p
                for h, av in ((0, av0), (1, av1)):
                    nc.tensor.matmul(
                        av[:, p_q0 - qc * 512:512],
                        vsb[:, p_kb, h, :], p_pt[:, h, 0:p_n],
                        start=(p_kb == 0), stop=last,
                        skip_group_check=True)

            for kb in range(nkb):
                k0 = kb * 128
                q0 = max(qc * 512, k0)
                q1 = qc * 512 + 512
                n = q1 - q0
                diag = (q0 == k0)
                sps = ps_s.tile([128, 1024], F32, tag="sps")
                for h in range(2):
                    nc.tensor.matmul(
                        sps[:, h * 512:h * 512 + n],
                        kt[h * 64:(h + 1) * 64, k0:k0 + 128],
                        qt[h * 64:(h + 1) * 64, q0:q1],
                        start=True, stop=True,
                        skip_group_check=True)
                pt = ptp.tile([128, 2, 512], BF16, tag="pt")
                nc.scalar.activation(
                    pt[:, :, 0:n],
                    sps[:].rearrange("p (h f) -> p h f", h=2)[:, :, 0:n],
                    AF.Exp)
                if diag:  # causal mask: zero the upper triangle post-exp
                    for h in range(2):
                        nc.vector.tensor_mul(pt[:, h, 0:128], pt[:, h, 0:128],
                                             c_tri01[:])
                if pend is not None:
                    emit_av(pend, last=False)
                pend = (pt, q0, n, kb)
                filler()  # drip one next-batch phase-A item into the gap
            emit_av(pend, last=True)

            # normalize both heads: denom row -> [128,4] via DMA so the exact
            # reciprocal runs across partitions (cost ~ cols), back to a
            # [1,512] row, broadcast on gpsimd, scale on DVE.
            qlo = qc * 512
            for h, av in ((0, av0), (1, av1)):
                rcps = nrm.tile([128, 2, 512], F32, tag="rcps")
                rsq = nrm.tile([128, 8], F32, tag="rsq")
                nc.vector.tensor_copy(rcps[64:65, 0, :], av[64:65, :])
                nc.gpsimd.dma_start(rsq[:, 0:4], rcps[64:65, 0, :])
                nc.vector.reciprocal(rsq[:, 4:8], rsq[:, 0:4])
                nc.gpsimd.dma_start(rcps[0:1, 1, :], rsq[:, 4:8])
                rb = nrm.tile([64, 512], F32, tag="rb")
                nc.gpsimd.partition_broadcast(rb[0:64, :], rcps[0:1, 1, :])
                if h == 0:
                    nc.vector.tensor_mul(attn[0:64, qlo:qlo + 512],
                                         av[0:64, :], rb[0:64, :])
                else:
                    a1 = nrm.tile([64, 512], BF16, tag="a1")
                    nc.vector.tensor_mul(a1[0:64, :], av[0:64, :],
                                         rb[0:64, :])
                    # scalar queue: phase-C's first LDW waits on this move,
                    # and the gpsimd queue is busy with swaps/normalize DMAs
                    nc.scalar.dma_start(attn[64:128, qlo:qlo + 512],
                                        a1[0:64, :])

        def emit_C_tp(tl, b, tp):
            # output projection for one 256-token group: both h5 halves into
            # one 2-bank pso, single drain per (tp, s), DVE/ACT alternating
            attn = tl["attn"]
            o_sb = osb.tile([128, 2, D], BF16, tag="osb")
            for s in range(2):
                tt = tp * 2 + s
                pso = ps_s.tile([128, 2, 512], F32, tag="sps")
                for h5 in range(2):
                    nc.tensor.matmul(pso[:, h5, :],
                                     attn[:, tt * 128:(tt + 1) * 128],
                                     w_o[:, h5 * 512:(h5 + 1) * 512],
                                     start=True, stop=True)
                src = pso[:].rearrange("p h f -> p (h f)")
                if s == 0:
                    nc.vector.tensor_copy(o_sb[:, s, :], src)
                else:
                    nc.scalar.activation(o_sb[:, s, :], src, AF.Copy)
            row = b * T + tp * 256
            nc.scalar.dma_start(
                out[row:row + 256, :].rearrange("(s p) f -> p s f", p=128),
                o_sb[:])

        def c_items(tl, b):
            return [lambda tl=tl, b=b, tp=tp: emit_C_tp(tl, b, tp)
                    for tp in range(8)]

        tiles = {0: alloc_batch(0)}
        for item in a_items(tiles[0]):
            item()
        pend_c = []  # previous batch's phase-C, dripped into this phase B
        for b in range(B):
            tl = tiles[b]
            attn_t = qkv.tile([128, T], BF16, tag="attnT")
            tl["attn"] = attn_t
            queue = list(pend_c)
            if b + 1 < B:
                tiles[b + 1] = alloc_batch(b + 1)
                queue = a_items(tiles[b + 1]) + queue
            qi = [0]

            def filler():
                if qi[0] < len(queue):
                    queue[qi[0]]()
                    qi[0] += 1

            for qc in range(4):
                emit_B_qc(tl, qc, filler)
            while qi[0] < len(queue):
                queue[qi[0]]()
                qi[0] += 1
            if b + 1 < B:
                pend_c = c_items(tl, b)
            else:
                for item in c_items(tl, b):
                    item()

    nc.compile()
    return nc


def _host_prep(x, W_qkv, W_o, token_positions):
    import ml_dtypes
    bf16 = ml_dtypes.bfloat16
    x = np.asarray(x, np.float32)
    W_qkv = np.asarray(W_qkv, np.float32)
    W_o = np.asarray(W_o, np.float32)
    pos = np.asarray(token_positions, np.float64)
    xT = np.ascontiguousarray(x.reshape(NT, D).T.astype(bf16))
    i = np.arange(32)
    inv = 1.0 / (ROPE_THETA ** (2 * i / DK))
    ang = pos[None, :] * inv[:, None]
    cos, sin = np.cos(ang), np.sin(ang)
    CC = np.tile(cos, (4, 1)).astype(np.float32)
    SS = np.concatenate([sin, -sin, sin, -sin], 0).astype(np.float32)
    trib = np.where(np.arange(128)[:, None] <= np.arange(128)[None, :],
                    1.0, 0.0).astype(bf16)
    idb = np.eye(128).astype(bf16)
    in_maps = []
    for c in range(NCORES):
        qcols, vcols = [], []
        for h in range(HPC):
            hh = HPC * c + h
            for half in range(2):
                qcols.extend(hh * DK + 2 * ii + half for ii in range(32))
            vcols.extend(hh * DK + d for d in range(DK))
        qcols = np.array(qcols)
        vcols = np.array(vcols)
        in_maps.append({
            "xT": xT,
            "wq": np.ascontiguousarray(W_qkv[:, 0 * D + qcols].astype(bf16)),
            "wk": np.ascontiguousarray((W_qkv[:, 1 * D + qcols] / 8.0).astype(bf16)),
            "wv": np.ascontiguousarray(W_qkv[:, 2 * D + vcols].astype(bf16)),
            "wo": np.ascontiguousarray(W_o[vcols, :].astype(bf16)),
            "cc": CC, "ss": SS, "trib": trib, "idb": idb,
        })
    return in_maps


def kernel(x, W_qkv, W_o, token_positions, _trace=False):
    in_maps = _host_prep(x, W_qkv, W_o, token_positions)
    if "nc" not in _BUILT:
        _BUILT["nc"] = _build_nc()
    res = run_bass_kernel_spmd(_BUILT["nc"], in_maps,
                               core_ids=list(range(NCORES)), trace=_trace)
    _BUILT["last_result"] = res
    total = np.zeros((NT, D), np.float32)
    for r in res.results:
        total += np.asarray(r["out"], dtype=np.float32)
    return total.reshape(B, T, D)
